# revision 56
# baseline (speedup 1.0000x reference)
"""Trainium2 Bass kernel for nn_K_attention_ex (gaussian-kernel residual attention).

Reference computation (per batch sample b):
    sq_i   = ||x_i||^2
    G      = x @ x^T                      (T,T) gram
    sqdist = relu(sq_i + sq_j - 2 G)
    K      = exp(-sqdist * r + m) * (1 - eye)
    out    = x + K @ x

Algebraic restructuring (exact up to fp rounding):
    K_full = beta * e_i * e_j * exp(2 r g_ij),   e = exp(-r*sq), beta = exp(m)
    out = (1-beta)*x + beta * e ⊙_row ( E @ (e ⊙_row x) ),  E = exp(2 r G)

E is symmetric, so only part of it is materialized. HW-measured costs
(not the cost model!) drove the split:
  * DMA-xbar transpose: ~1.65us fixed per instruction + ~27ns per 16x128
    tile -> mirroring all 120 lower blocks via xbar is ~50us/sample. Only
    columns j < CX are mirrored via xbar (the widest strips, fewest
    instructions per block).
  * ACT exp: ~1.2-2ns/elem on HW. Rows a > CX recompute their lower
    columns [CX, a) directly: each stored E row-strip covers columns
    [min(a,CX), 16), so the Y matmul for stationary xs_a consumes ONE
    contiguous wide rhs (upper + recomputed-lower merged) plus CX small
    mirror matmuls from the xbar'd ET blocks.
  * PE: bf16 matmuls 1cyc/col; rotating 64-col stationaries are ~hidden
    (measured 264ns per N=512 matmul). Wide-N everywhere where possible.
  * Y^T -> natural layout via 2 DMA-xbar transposes; PSUM evacuations on
    DVE (GPSIMD cannot touch PSUM on HW); combine on DVE.

Both samples run in ONE merged software pipeline over global steps
(sample stream offset SOFF), hiding each sample's fill/drain behind the
other's compute.

bf16 operands for gram + Y (output rel err ~4e-3 vs the 2e-2 gate); fp8
gram was rejected: per-row quantization error of x is amplified by the
near-constant positive E into ~3e-2 output error.

Sharding: data-parallel over batch B=16 across 8 NeuronCores (2 samples each).
"""

import numpy as np

import concourse.bass as bass
import concourse.tile as tile
from concourse import bacc, mybir
from concourse.bass_utils import run_bass_kernel_spmd
from concourse.masks import make_identity

F32 = mybir.dt.float32
BF16 = mybir.dt.bfloat16
FP8 = mybir.dt.float8e4
DR = mybir.MatmulPerfMode.DoubleRow
AF = mybir.ActivationFunctionType
MUL = mybir.AluOpType.mult
ADD = mybir.AluOpType.add
SUB = mybir.AluOpType.subtract

B, T, C = 16, 2048, 64
N_CORES = 8
BPC = B // N_CORES          # samples per core
NK = T // 128               # 16 row-blocks of 128
YSHIFT = 2                  # Y stationary lag behind exp/xbar
SOFF = NK - 3               # step offset between sample pipelines (overlap=3)
CX = 4                      # columns j < CX mirrored via DMA-xbar (multiple of 4)

# E storage: row a holds columns [C0[a], 16) contiguously at EOF2[a].
C0 = [min(a, CX) for a in range(NK)]
WB = [NK - C0[a] for a in range(NK)]          # width in 128-col blocks
EOF2 = []
_o = 0
for _a in range(NK):
    EOF2.append(_o)
    _o += WB[_a] * 128
E_W = _o

# ET: transposed off-diag blocks (rows j < CX only): block (row j, col k>j)
# lands at slot ET_OFF2[j] + (k-j-1); Y stationary a reads slot for (j, a).
ET_OFF2 = []
_o = 0
for _j in range(CX):
    ET_OFF2.append(_o)
    _o += (NK - 1) - _j
N_ET = _o


def build_nc(reps=1, stages='all'):
    nc = bacc.Bacc("TRN2", target_bir_lowering=False, debug=False, num_devices=N_CORES)
    x_in = nc.dram_tensor("x", [BPC, T, C], F32, kind="ExternalInput")
    r_in = nc.dram_tensor("r_sigma", [1], F32, kind="ExternalInput")
    m_in = nc.dram_tensor("margin", [1], F32, kind="ExternalInput")
    o_out = nc.dram_tensor("out", [BPC, T, C], F32, kind="ExternalOutput")

    with tile.TileContext(nc) as tc:
        if reps == 1:
            _body(tc, o_out.ap(), x_in.ap(), r_in.ap(), m_in.ap(), stages)
        else:
            with tc.For_i(0, reps, 1):
                _body(tc, o_out.ap(), x_in.ap(), r_in.ap(), m_in.ap(), stages)
    nc.compile()
    return nc


LEVELS = {'xload': 0, 'xt': 1, 'prep': 2, 'gram': 3, 'exp': 4, 'xbar': 5, 'y': 6, 'all': 7}


def _body(tc, out_ap, x_ap, r_ap, m_ap, stages='all'):
    lvl = LEVELS[stages]
    do = lambda name: lvl >= LEVELS.get(name, 7)
    nc = tc.nc
    with (
        tc.tile_pool(name="consts", bufs=1) as consts,
        tc.tile_pool(name="sx", bufs=2) as sx,
        tc.tile_pool(name="ebig", bufs=2) as ebig,
        tc.tile_pool(name="psG", bufs=2, space="PSUM") as psG,
        tc.tile_pool(name="psY", bufs=2, space="PSUM") as psY,
    ):
        # ---- one-time constants ----
        identb = consts.tile([128, 128], BF16)
        make_identity(nc, identb)
        rb = consts.tile([128, 1], F32)
        nc.gpsimd.dma_start(out=rb, in_=r_ap.to_broadcast((128, 1)))
        mb = consts.tile([128, 1], F32)
        nc.gpsimd.dma_start(out=mb, in_=m_ap.to_broadcast((128, 1)))
        negr = consts.tile([128, 1], F32)
        nc.vector.tensor_scalar_mul(out=negr, in0=rb, scalar1=-1.0)
        s2r = consts.tile([128, 1], F32)
        nc.vector.tensor_scalar_mul(out=s2r, in0=rb, scalar1=2.0)
        beta = consts.tile([128, 1], F32)
        nc.scalar.activation(out=beta, in_=mb, func=AF.Exp)
        alpha = consts.tile([128, 1], F32)  # 1 - beta
        nc.vector.tensor_scalar(
            out=alpha, in0=beta, scalar1=-1.0, scalar2=1.0, op0=MUL, op1=ADD,
        )

        # prefetch all samples' inputs up front
        x_sbs = []
        for s in range(BPC):
            xv = x_ap[s].rearrange("(p k) c -> p k c", p=128)
            x_sb = sx.tile([128, NK, C], F32, tag="x_sb", name=f"x_sb_{s}")
            nc.sync.dma_start(out=x_sb[:, 0:8, :], in_=xv[:, 0:8, :])
            nc.gpsimd.dma_start(out=x_sb[:, 8:NK, :], in_=xv[:, 8:NK, :])
            x_sbs.append(x_sb)

        # ---- per-sample state ----
        st = [
            {
                "x_bf": None, "xTb": None, "xsq": None, "sq": None,
                "f": None, "xs": None, "ax": None,
                "E": None, "ET": None, "YTsb": None, "ynat": None,
                "outsb": None, "yt": [None] * 4, "gs": None,
                "xT8": None, "dT": None, "x8n": None, "dn": None,
                "Pc": None, "Qc": None,
            }
            for _ in range(BPC)
        ]
        out_legs = [nc.sync, nc.gpsimd]

        # ---- front-end emitters ----
        def emit_sq_mul(s):
            if not do('prep'):
                return
            xsq = sx.tile([128, NK, C], F32, tag="xsq", name=f"xsq_{s}")
            nc.vector.tensor_mul(xsq, x_sbs[s], x_sbs[s])
            st[s]["xsq"] = xsq

        def emit_sq_reduce(s):
            if not do('prep'):
                return
            sq = sx.tile([128, NK], F32, tag="sq", name=f"sq_{s}")
            nc.vector.reduce_sum(out=sq, in_=st[s]["xsq"], axis=mybir.AxisListType.X)
            st[s]["sq"] = sq

        def emit_cast(s):
            x_bf = sx.tile([128, NK, C], BF16, tag="x_bf", name=f"x_bf_{s}")
            # two halves, following the two input-DMA legs (subtile deps)
            nc.vector.tensor_copy(out=x_bf[:, 0:8, :], in_=x_sbs[s][:, 0:8, :])
            nc.vector.tensor_copy(out=x_bf[:, 8:NK, :], in_=x_sbs[s][:, 8:NK, :])
            st[s]["x_bf"] = x_bf
            st[s]["xTb"] = sx.tile([64, T], BF16, tag="xTb", name=f"xTb_{s}")

        def emit_transp(s, g):
            if not do('xt'):
                return
            x_bf, xTb = st[s]["x_bf"], st[s]["xTb"]
            if g == 0:
                st[s]["xT8"] = sx.tile([64, 2, T], FP8, tag="xT8", name=f"xT8_{s}")
                nc.gpsimd.memset(st[s]["xT8"][:, 1, :], 0.0)
                st[s]["dT"] = sx.tile([64, T], BF16, tag="dT", name=f"dT_{s}")
            xtr = psG.tile([64, 4, 128], BF16, tag="G", name=f"xtr_{s}_{g}")
            for kk in range(4):
                k = 4 * g + kk
                nc.tensor.transpose(
                    out=xtr[:, kk, :], in_=x_bf[:, k, :], identity=identb
                )
            sl = slice(512 * g, 512 * (g + 1))
            nc.vector.tensor_copy(
                out=xTb[:, sl], in_=xtr.rearrange("p a b -> p (a b)")
            )
            # fp8 mirror of x^T (DoubleRow gram operand) + bf16 residual
            nc.vector.tensor_copy(
                out=st[s]["xT8"][:, 0, sl], in_=xtr.rearrange("p a b -> p (a b)")
            )
            nc.vector.tensor_sub(st[s]["dT"][:, sl], xTb[:, sl], st[s]["xT8"][:, 0, sl])

        def emit_xsf(s):
            if not do('prep'):
                return
            e = sx.tile([128, NK], F32, tag="e", name=f"e_{s}")
            nc.scalar.activation(out=e, in_=st[s]["sq"], func=AF.Exp, scale=negr)
            f = sx.tile([128, NK], F32, tag="f", name=f"f_{s}")
            nc.vector.tensor_scalar_mul(out=f, in0=e, scalar1=beta)
            xs_bf = sx.tile([128, NK, C], BF16, tag="xs_bf", name=f"xs_bf_{s}")
            for k in range(NK):
                nc.vector.tensor_scalar_mul(
                    out=xs_bf[:, k, :], in0=st[s]["x_bf"][:, k, :],
                    scalar1=e[:, k : k + 1],
                )
            st[s]["f"], st[s]["xs"] = f, xs_bf

        def emit_ax(s):
            if not do('prep'):
                return
            ax = sx.tile([128, NK, C], F32, tag="ax", name=f"ax_{s}")
            nc.vector.tensor_scalar_mul(out=ax, in0=x_sbs[s], scalar1=alpha)
            st[s]["ax"] = ax

        def emit_corr(s):
            # Rank-64 correction for the fp8 gram: with d = x - fp8(x),
            #   Y_true - Y_fp8 ~= 2r * (x @ Q + d @ (P - Q)),
            #   P = x^T xs,  Q = d^T xs   (64x64 each)
            # This cancels the systematic per-row quantization error that
            # the near-constant positive E amplifies (2.6e-2 -> 4.6e-3).
            if not do('y'):
                return
            x_bf, xs_bf = st[s]["x_bf"], st[s]["xs"]
            x8n = sx.tile([128, NK, C], FP8, tag="x8n", name=f"x8n_{s}", bufs=1)
            nc.vector.tensor_copy(out=x8n, in_=x_bf)
            dn = sx.tile([128, NK, C], BF16, tag="dn", name=f"dn_{s}", bufs=1)
            nc.vector.tensor_sub(dn, x_bf, x8n)
            P = psG.tile([64, 64], F32, tag="G", name=f"P_{s}")
            for k in range(NK):
                nc.tensor.matmul(
                    out=P, lhsT=x_bf[:, k, :], rhs=xs_bf[:, k, :],
                    start=(k == 0), stop=(k == NK - 1),
                )
            Q = psG.tile([64, 64], F32, tag="G", name=f"Q_{s}")
            for k in range(NK):
                nc.tensor.matmul(
                    out=Q, lhsT=dn[:, k, :], rhs=xs_bf[:, k, :],
                    start=(k == 0), stop=(k == NK - 1),
                )
            Qc = sx.tile([64, 64], BF16, tag="Qc", name=f"Qc_{s}")
            nc.vector.tensor_scalar_mul(out=Qc, in0=Q, scalar1=s2r[:64])
            Pc = sx.tile([64, 64], BF16, tag="Pc", name=f"Pc_{s}")
            nc.vector.scalar_tensor_tensor(
                out=Pc, in0=P, scalar=s2r[:64], in1=Qc, op0=MUL, op1=SUB,
            )
            st[s]["Pc"], st[s]["Qc"] = Pc, Qc

        def emit_front(s):
            emit_cast(s)
            emit_transp(s, 0)
            emit_transp(s, 1)
            emit_sq_mul(s)
            emit_sq_reduce(s)
            emit_transp(s, 2)
            emit_transp(s, 3)
            emit_xsf(s)

        def alloc_main(s):
            st[s]["E"] = ebig.tile([128, E_W], BF16, tag="E", name=f"E_{s}")
            st[s]["ET"] = ebig.tile(
                [128, N_ET, 128], BF16, tag="ET", name=f"ET_{s}", bufs=1
            )
            st[s]["YTsb"] = sx.tile([64, T], BF16, tag="YTsb", name=f"YTsb_{s}")
            st[s]["ynat"] = sx.tile([128, NK, C], BF16, tag="ynat", name=f"ynat_{s}")
            st[s]["outsb"] = sx.tile([128, NK, C], F32, tag="outsb", name=f"outsb_{s}")

        # ---- main-pipeline emitters ----
        def emit_gram(s, a):
            """G row-strip a (fp8 DoubleRow, 0.5 PE cyc/col) over columns
            [128*C0[a], 2048).

            1536-col G tiles (3 bank-sized sub-matmuls each) so the exp
            reads 1536-wide chunks: ACT per-instruction overhead measured
            ~650ns, so wide chunks substantially cut the HW exp cost."""
            xT8 = st[s]["xT8"]
            lhsT = xT8[:, :, 128 * a : 128 * (a + 1)]
            gs = []
            c0 = 128 * C0[a]
            while c0 < T:
                w = min(1536, T - c0)
                G = psG.tile([128, 1536], F32, tag="G", name=f"G_{s}_{a}_{c0}")
                for q0 in range(0, w, 512):
                    qw = min(512, w - q0)
                    nc.tensor.matmul(
                        out=G[:, q0 : q0 + qw],
                        lhsT=lhsT,
                        rhs=xT8[:, :, c0 + q0 : c0 + q0 + qw],
                        start=True,
                        stop=True,
                        perf_mode=DR,
                    )
                gs.append((G, c0, w))
                c0 += w
            st[s]["gs"] = gs

        def emit_exp(s, a):
            E = st[s]["E"]
            for (G, c0, w) in st[s]["gs"]:
                o0 = EOF2[a] + (c0 - 128 * C0[a])
                if do('exp'):
                    nc.scalar.activation(
                        out=E[:, o0 : o0 + w], in_=G[:, 0:w], func=AF.Exp, scale=s2r,
                    )
                else:
                    nc.scalar.activation(
                        out=E[:, o0 : o0 + w], in_=G[:, 0:w], func=AF.Copy,
                    )

        def emit_xbar(s, j):
            # mirror row j's off-diag blocks (cols j+1..15) for columns j<CX
            if j >= CX or not do('xbar'):
                return
            E, ET = st[s]["E"], st[s]["ET"]
            nb = (NK - 1) - j
            nc.sync.dma_start_transpose(
                out=ET[:, ET_OFF2[j] : ET_OFF2[j] + nb, :],
                in_=E[:, EOF2[j] + 128 : EOF2[j] + (NK - j) * 128],
            )

        def emit_ya(s, a, qp):
            # Y^T quarter-pass qp (cols [512*qp, 512*(qp+1))) for stationary
            # xs_a. Quarter 0 carries all the xbar-mirror columns (CX=4);
            # quarters use 1 PSUM bank each and run lag-staggered so only
            # two are ever live (pool rotation enforces it).
            E, ET, xs_bf = st[s]["E"], st[s]["ET"], st[s]["xs"]
            yts = st[s]["yt"]
            if yts[qp] is None:
                yts[qp] = psY.tile([64, 512], F32, tag="YT", name=f"YT_{s}_{qp}")
            yt = yts[qp]
            lhsT = xs_bf[:, a, :]
            last = a == NK - 1
            q0 = 512 * qp

            if qp == 0:
                # mirror matmuls (cols j < min(a, CX) <= 4, all in quarter 0)
                for j in range(min(a, CX)):
                    nc.tensor.matmul(
                        out=yt[:, 128 * j : 128 * (j + 1)],
                        lhsT=lhsT,
                        rhs=ET[:, ET_OFF2[j] + (a - j - 1), :],
                        start=False,
                        stop=last and j == CX - 1 and C0[a] >= 4,
                    )
            # strip part inside this quarter: cols [max(q0, 128*C0[a]), q0+512)
            c0 = max(q0, 128 * C0[a])
            w = q0 + 512 - c0
            if w > 0:
                o = EOF2[a] + (c0 - 128 * C0[a])
                nc.tensor.matmul(
                    out=yt[:, c0 - q0 : c0 - q0 + w],
                    lhsT=lhsT,
                    rhs=E[:, o : o + w],
                    start=(a == 0),
                    stop=last,
                )
            if a == 0:
                # fp8-gram correction terms: += x @ Qc + d @ Pc (this quarter)
                nc.tensor.matmul(
                    out=yt, lhsT=st[s]["Qc"], rhs=st[s]["xTb"][:, q0 : q0 + 512],
                    start=False, stop=False,
                )
                nc.tensor.matmul(
                    out=yt, lhsT=st[s]["Pc"], rhs=st[s]["dT"][:, q0 : q0 + 512],
                    start=False, stop=False,
                )
            if last:
                nc.vector.tensor_copy(
                    out=st[s]["YTsb"][:, q0 : q0 + 512], in_=yt
                )
                yts[qp] = None

        def emit_out(s, g):
            # half g: Y^T 1024-col slab -> natural layout, combine, store
            if not do('all'):
                return
            YTsb, ynat, outsb = st[s]["YTsb"], st[s]["ynat"], st[s]["outsb"]
            f, ax = st[s]["f"], st[s]["ax"]
            nc.sync.dma_start_transpose(
                out=ynat[:, 8 * g : 8 * (g + 1), :],
                in_=YTsb[:, 1024 * g : 1024 * (g + 1)],
            )
            for k in range(8 * g, 8 * (g + 1)):
                nc.vector.scalar_tensor_tensor(
                    out=outsb[:, k, :], in0=ynat[:, k, :], scalar=f[:, k : k + 1],
                    in1=ax[:, k, :], op0=MUL, op1=ADD,
                )
            ov = out_ap[s].rearrange("(p k) c -> p k c", p=128)
            out_legs[g].dma_start(
                out=ov[:, 8 * g : 8 * (g + 1), :], in_=outsb[:, 8 * g : 8 * (g + 1), :]
            )

        # ---- merged pipeline over all samples ----
        emit_front(0)

        if not do('gram'):
            for s in range(1, BPC):
                emit_front(s)
            if do('all'):
                for s in range(BPC):
                    emit_ax(s)
                    st[s]["YTsb"] = sx.tile([64, T], BF16, tag="YTsb", name=f"YTsb_{s}")
                    st[s]["ynat"] = sx.tile([128, NK, C], BF16, tag="ynat", name=f"ynat_{s}")
                    st[s]["outsb"] = sx.tile([128, NK, C], F32, tag="outsb", name=f"outsb_{s}")
                    nc.vector.memset(st[s]["YTsb"], 0.0)
                    for g in range(2):
                        emit_out(s, g)
            return

        QLAG = 9    # step lag between Y quarter-passes (PSUM: <=2 live)
        JMAX = NK - 1 + YSHIFT + 3 * QLAG + 2
        T_END = (BPC - 1) * SOFF + JMAX
        for t in range(-1, T_END + 1):
            for s in range(BPC):
                j = t - s * SOFF
                if j < -1 or j > JMAX:
                    continue
                if j == -1:
                    alloc_main(s)
                    emit_gram(s, 0)
                    continue
                if j < NK:
                    emit_exp(s, j)
                if do('y'):
                    for qp in range(4):
                        ay = j - YSHIFT - QLAG * qp
                        if 0 <= ay < NK:
                            emit_ya(s, ay, qp)
                jg = j + 1
                if jg < NK:
                    emit_gram(s, jg)
                if j < NK:
                    emit_xbar(s, j)
                # hooks: per-sample corr/ax + next sample's front-end
                if j == 0:
                    emit_corr(s)
                elif j == 1:
                    emit_ax(s)
                sn = s + 1
                if sn < BPC:
                    if j == SOFF - 9:
                        emit_cast(sn)
                    elif SOFF - 8 <= j <= SOFF - 5:
                        emit_transp(sn, j - (SOFF - 8))
                        if j == SOFF - 8:
                            emit_sq_mul(sn)
                        elif j == SOFF - 7:
                            emit_sq_reduce(sn)
                    elif j == SOFF - 4:
                        emit_xsf(sn)
                # output halves: half g complete after quarter 2g+1's drain
                if do('y'):
                    if j == NK + YSHIFT + QLAG + 1:
                        emit_out(s, 0)
                    elif j == NK + YSHIFT + 3 * QLAG + 1:
                        emit_out(s, 1)

        if do('all') and not do('y'):
            for s in range(BPC):
                nc.vector.memset(st[s]["YTsb"], 0.0)
                for g in range(2):
                    emit_out(s, g)


_NC_CACHE = {}


def _get_nc(reps=1, stages='all'):
    key = (reps, stages)
    if key not in _NC_CACHE:
        _NC_CACHE[key] = build_nc(reps, stages)
    return _NC_CACHE[key]


def _run(x, r_sigma, margin, trace=False, reps=1, stages='all'):
    nc = _get_nc(reps, stages)
    x = np.ascontiguousarray(np.asarray(x, dtype=np.float32))
    r_sigma = np.ascontiguousarray(np.asarray(r_sigma, dtype=np.float32))
    margin = np.ascontiguousarray(np.asarray(margin, dtype=np.float32))
    in_maps = [
        {
            "x": np.ascontiguousarray(x[c * BPC : (c + 1) * BPC]),
            "r_sigma": r_sigma,
            "margin": margin,
        }
        for c in range(N_CORES)
    ]
    res = run_bass_kernel_spmd(nc, in_maps, core_ids=list(range(N_CORES)), trace=trace)
    out = np.concatenate([res.results[c]["out"] for c in range(N_CORES)], axis=0)
    return out, res


def kernel(x, r_sigma, margin):
    out, _ = _run(x, r_sigma, margin, trace=False)
    return out


# revision 62
# speedup vs baseline: 1.2598x; 1.2598x over previous
"""Trainium2 Bass kernel for nn_K_attention_ex (gaussian-kernel residual attention).

Reference computation (per batch sample b):
    sq_i   = ||x_i||^2
    G      = x @ x^T                      (T,T) gram
    sqdist = relu(sq_i + sq_j - 2 G)
    K      = exp(-sqdist * r + m) * (1 - eye)
    out    = x + K @ x

Algebraic restructuring (exact up to fp rounding):
    K_full = beta * e_i * e_j * exp(2 r g_ij),   e = exp(-r*sq), beta = exp(m)
    out = (1-beta)*x + beta * e ⊙_row ( E @ (e ⊙_row x) ),  E = exp(2 r G)

E is symmetric, so only part of it is materialized. HW-measured costs
(not the cost model!) drove the split:
  * DMA-xbar transpose: ~1.65us fixed per instruction + ~27ns per 16x128
    tile -> mirroring all 120 lower blocks via xbar is ~50us/sample. Only
    columns j < CX are mirrored via xbar (the widest strips, fewest
    instructions per block).
  * ACT exp: ~1.2-2ns/elem on HW. Rows a > CX recompute their lower
    columns [CX, a) directly: each stored E row-strip covers columns
    [min(a,CX), 16), so the Y matmul for stationary xs_a consumes ONE
    contiguous wide rhs (upper + recomputed-lower merged) plus CX small
    mirror matmuls from the xbar'd ET blocks.
  * PE: bf16 matmuls 1cyc/col; rotating 64-col stationaries are ~hidden
    (measured 264ns per N=512 matmul). Wide-N everywhere where possible.
  * Y^T -> natural layout via 2 DMA-xbar transposes; PSUM evacuations on
    DVE (GPSIMD cannot touch PSUM on HW); combine on DVE.

Both samples run in ONE merged software pipeline over global steps
(sample stream offset SOFF), hiding each sample's fill/drain behind the
other's compute.

bf16 operands for gram + Y (output rel err ~4e-3 vs the 2e-2 gate); fp8
gram was rejected: per-row quantization error of x is amplified by the
near-constant positive E into ~3e-2 output error.

Sharding: data-parallel over batch B=16 across 8 NeuronCores (2 samples each).
"""

import numpy as np

import concourse.bass as bass
import concourse.tile as tile
from concourse import bacc, mybir
from concourse.bass_utils import run_bass_kernel_spmd
from concourse.masks import make_identity

F32 = mybir.dt.float32
BF16 = mybir.dt.bfloat16
FP8 = mybir.dt.float8e4
DR = mybir.MatmulPerfMode.DoubleRow
AF = mybir.ActivationFunctionType
MUL = mybir.AluOpType.mult
ADD = mybir.AluOpType.add
SUB = mybir.AluOpType.subtract

B, T, C = 16, 2048, 64
N_CORES = 8
BPC = B // N_CORES          # samples per core
NK = T // 128               # 16 row-blocks of 128
YSHIFT = 2                  # Y stationary lag behind exp/xbar
SOFF = NK - 3               # step offset between sample pipelines (overlap=3)
CX = 4                      # columns j < CX mirrored via DMA-xbar (multiple of 4)
GRAM_FP8 = False            # fp8-DoubleRow gram + rank-64 correction: left
                            # implemented but OFF — measured slower on HW
                            # (DR gave no real speedup; bf16 err is fine)

# E storage: row a holds columns [C0[a], 16) contiguously at EOF2[a].
C0 = [min(a, CX) for a in range(NK)]
WB = [NK - C0[a] for a in range(NK)]          # width in 128-col blocks
EOF2 = []
_o = 0
for _a in range(NK):
    EOF2.append(_o)
    _o += WB[_a] * 128
E_W = _o

# ET: transposed off-diag blocks (rows j < CX only): block (row j, col k>j)
# lands at slot ET_OFF2[j] + (k-j-1); Y stationary a reads slot for (j, a).
ET_OFF2 = []
_o = 0
for _j in range(CX):
    ET_OFF2.append(_o)
    _o += (NK - 1) - _j
N_ET = _o


def build_nc(reps=1, stages='all'):
    nc = bacc.Bacc("TRN2", target_bir_lowering=False, debug=False, num_devices=N_CORES)
    x_in = nc.dram_tensor("x", [BPC, T, C], F32, kind="ExternalInput")
    r_in = nc.dram_tensor("r_sigma", [1], F32, kind="ExternalInput")
    m_in = nc.dram_tensor("margin", [1], F32, kind="ExternalInput")
    o_out = nc.dram_tensor("out", [BPC, T, C], F32, kind="ExternalOutput")

    with tile.TileContext(nc) as tc:
        if reps == 1:
            _body(tc, o_out.ap(), x_in.ap(), r_in.ap(), m_in.ap(), stages)
        else:
            with tc.For_i(0, reps, 1):
                _body(tc, o_out.ap(), x_in.ap(), r_in.ap(), m_in.ap(), stages)
    nc.compile()
    return nc


LEVELS = {'xload': 0, 'xt': 1, 'prep': 2, 'gram': 3, 'exp': 4, 'xbar': 5, 'y': 6, 'all': 7}


def _body(tc, out_ap, x_ap, r_ap, m_ap, stages='all'):
    lvl = LEVELS[stages]
    do = lambda name: lvl >= LEVELS.get(name, 7)
    nc = tc.nc
    with (
        tc.tile_pool(name="consts", bufs=1) as consts,
        tc.tile_pool(name="sx", bufs=2) as sx,
        tc.tile_pool(name="ebig", bufs=2) as ebig,
        tc.tile_pool(name="psG", bufs=2, space="PSUM") as psG,
        tc.tile_pool(name="psY", bufs=2, space="PSUM") as psY,
    ):
        # ---- one-time constants ----
        identb = consts.tile([128, 128], BF16)
        make_identity(nc, identb)
        rb = consts.tile([128, 1], F32)
        nc.gpsimd.dma_start(out=rb, in_=r_ap.to_broadcast((128, 1)))
        mb = consts.tile([128, 1], F32)
        nc.gpsimd.dma_start(out=mb, in_=m_ap.to_broadcast((128, 1)))
        negr = consts.tile([128, 1], F32)
        nc.vector.tensor_scalar_mul(out=negr, in0=rb, scalar1=-1.0)
        s2r = consts.tile([128, 1], F32)
        nc.vector.tensor_scalar_mul(out=s2r, in0=rb, scalar1=2.0)
        beta = consts.tile([128, 1], F32)
        nc.scalar.activation(out=beta, in_=mb, func=AF.Exp)
        alpha = consts.tile([128, 1], F32)  # 1 - beta
        nc.vector.tensor_scalar(
            out=alpha, in0=beta, scalar1=-1.0, scalar2=1.0, op0=MUL, op1=ADD,
        )

        # prefetch all samples' inputs up front
        x_sbs = []
        for s in range(BPC):
            xv = x_ap[s].rearrange("(p k) c -> p k c", p=128)
            x_sb = sx.tile([128, NK, C], F32, tag="x_sb", name=f"x_sb_{s}")
            nc.sync.dma_start(out=x_sb[:, 0:8, :], in_=xv[:, 0:8, :])
            nc.gpsimd.dma_start(out=x_sb[:, 8:NK, :], in_=xv[:, 8:NK, :])
            x_sbs.append(x_sb)

        # ---- per-sample state ----
        st = [
            {
                "x_bf": None, "xTb": None, "xsq": None, "sq": None,
                "f": None, "xs": None, "ax": None,
                "E": None, "ET": None, "YTsb": None, "ynat": None,
                "outsb": None, "yt": [None] * 4, "gs": None,
                "xT8": None, "dT": None, "x8n": None, "dn": None,
                "Pc": None, "Qc": None,
            }
            for _ in range(BPC)
        ]
        out_legs = [nc.sync, nc.gpsimd]

        # ---- front-end emitters ----
        def emit_sq_mul(s):
            if not do('prep'):
                return
            xsq = sx.tile([128, NK, C], F32, tag="xsq", name=f"xsq_{s}")
            nc.vector.tensor_mul(xsq, x_sbs[s], x_sbs[s])
            st[s]["xsq"] = xsq

        def emit_sq_reduce(s):
            if not do('prep'):
                return
            sq = sx.tile([128, NK], F32, tag="sq", name=f"sq_{s}")
            nc.vector.reduce_sum(out=sq, in_=st[s]["xsq"], axis=mybir.AxisListType.X)
            st[s]["sq"] = sq

        def emit_cast(s):
            x_bf = sx.tile([128, NK, C], BF16, tag="x_bf", name=f"x_bf_{s}")
            # two halves, following the two input-DMA legs (subtile deps)
            nc.vector.tensor_copy(out=x_bf[:, 0:8, :], in_=x_sbs[s][:, 0:8, :])
            nc.vector.tensor_copy(out=x_bf[:, 8:NK, :], in_=x_sbs[s][:, 8:NK, :])
            st[s]["x_bf"] = x_bf
            st[s]["xTb"] = sx.tile([64, T], BF16, tag="xTb", name=f"xTb_{s}")

        def emit_transp(s, g):
            if not do('xt'):
                return
            x_bf, xTb = st[s]["x_bf"], st[s]["xTb"]
            if g == 0 and GRAM_FP8:
                st[s]["xT8"] = sx.tile([64, 2, T], FP8, tag="xT8", name=f"xT8_{s}")
                nc.gpsimd.memset(st[s]["xT8"][:, 1, :], 0.0)
                st[s]["dT"] = sx.tile([64, T], BF16, tag="dT", name=f"dT_{s}")
            xtr = psG.tile([64, 4, 128], BF16, tag="G", name=f"xtr_{s}_{g}")
            for kk in range(4):
                k = 4 * g + kk
                nc.tensor.transpose(
                    out=xtr[:, kk, :], in_=x_bf[:, k, :], identity=identb
                )
            sl = slice(512 * g, 512 * (g + 1))
            nc.vector.tensor_copy(
                out=xTb[:, sl], in_=xtr.rearrange("p a b -> p (a b)")
            )
            if GRAM_FP8:
                # fp8 mirror of x^T (DoubleRow gram operand) + bf16 residual
                nc.vector.tensor_copy(
                    out=st[s]["xT8"][:, 0, sl], in_=xtr.rearrange("p a b -> p (a b)")
                )
                nc.vector.tensor_sub(
                    st[s]["dT"][:, sl], xTb[:, sl], st[s]["xT8"][:, 0, sl]
                )

        def emit_xsf(s):
            if not do('prep'):
                return
            e = sx.tile([128, NK], F32, tag="e", name=f"e_{s}")
            nc.scalar.activation(out=e, in_=st[s]["sq"], func=AF.Exp, scale=negr)
            f = sx.tile([128, NK], F32, tag="f", name=f"f_{s}")
            nc.vector.tensor_scalar_mul(out=f, in0=e, scalar1=beta)
            xs_bf = sx.tile([128, NK, C], BF16, tag="xs_bf", name=f"xs_bf_{s}")
            for k in range(NK):
                nc.vector.tensor_scalar_mul(
                    out=xs_bf[:, k, :], in0=st[s]["x_bf"][:, k, :],
                    scalar1=e[:, k : k + 1],
                )
            st[s]["f"], st[s]["xs"] = f, xs_bf

        def emit_ax(s):
            if not do('prep'):
                return
            ax = sx.tile([128, NK, C], F32, tag="ax", name=f"ax_{s}")
            nc.vector.tensor_scalar_mul(out=ax, in0=x_sbs[s], scalar1=alpha)
            st[s]["ax"] = ax

        def emit_corr(s):
            # Rank-64 correction for the fp8 gram: with d = x - fp8(x),
            #   Y_true - Y_fp8 ~= 2r * (x @ Q + d @ (P - Q)),
            #   P = x^T xs,  Q = d^T xs   (64x64 each)
            # This cancels the systematic per-row quantization error that
            # the near-constant positive E amplifies (2.6e-2 -> 4.6e-3).
            if not do('y') or not GRAM_FP8:
                return
            x_bf, xs_bf = st[s]["x_bf"], st[s]["xs"]
            x8n = sx.tile([128, NK, C], FP8, tag="x8n", name=f"x8n_{s}", bufs=1)
            nc.vector.tensor_copy(out=x8n, in_=x_bf)
            dn = sx.tile([128, NK, C], BF16, tag="dn", name=f"dn_{s}", bufs=1)
            nc.vector.tensor_sub(dn, x_bf, x8n)
            P = psG.tile([64, 64], F32, tag="G", name=f"P_{s}")
            for k in range(NK):
                nc.tensor.matmul(
                    out=P, lhsT=x_bf[:, k, :], rhs=xs_bf[:, k, :],
                    start=(k == 0), stop=(k == NK - 1),
                )
            Q = psG.tile([64, 64], F32, tag="G", name=f"Q_{s}")
            for k in range(NK):
                nc.tensor.matmul(
                    out=Q, lhsT=dn[:, k, :], rhs=xs_bf[:, k, :],
                    start=(k == 0), stop=(k == NK - 1),
                )
            Qc = sx.tile([64, 64], BF16, tag="Qc", name=f"Qc_{s}")
            nc.vector.tensor_scalar_mul(out=Qc, in0=Q, scalar1=s2r[:64])
            Pc = sx.tile([64, 64], BF16, tag="Pc", name=f"Pc_{s}")
            nc.vector.scalar_tensor_tensor(
                out=Pc, in0=P, scalar=s2r[:64], in1=Qc, op0=MUL, op1=SUB,
            )
            st[s]["Pc"], st[s]["Qc"] = Pc, Qc

        def emit_front(s):
            emit_cast(s)
            emit_transp(s, 0)
            emit_transp(s, 1)
            emit_sq_mul(s)
            emit_sq_reduce(s)
            emit_transp(s, 2)
            emit_transp(s, 3)
            emit_xsf(s)

        def alloc_main(s):
            st[s]["E"] = ebig.tile([128, E_W], BF16, tag="E", name=f"E_{s}")
            st[s]["ET"] = ebig.tile(
                [128, N_ET, 128], BF16, tag="ET", name=f"ET_{s}", bufs=1
            )
            st[s]["YTsb"] = sx.tile([64, T], BF16, tag="YTsb", name=f"YTsb_{s}")
            st[s]["ynat"] = sx.tile([128, NK, C], BF16, tag="ynat", name=f"ynat_{s}")
            st[s]["outsb"] = sx.tile([128, NK, C], F32, tag="outsb", name=f"outsb_{s}")

        # ---- main-pipeline emitters ----
        def emit_gram(s, a):
            """G row-strip a (fp8 DoubleRow, 0.5 PE cyc/col) over columns
            [128*C0[a], 2048).

            1536-col G tiles (3 bank-sized sub-matmuls each) so the exp
            reads 1536-wide chunks: ACT per-instruction overhead measured
            ~650ns, so wide chunks substantially cut the HW exp cost."""
            gs = []
            c0 = 128 * C0[a]
            while c0 < T:
                w = min(1536, T - c0)
                G = psG.tile([128, 1536], F32, tag="G", name=f"G_{s}_{a}_{c0}")
                for q0 in range(0, w, 512):
                    qw = min(512, w - q0)
                    if GRAM_FP8:
                        xT8 = st[s]["xT8"]
                        nc.tensor.matmul(
                            out=G[:, q0 : q0 + qw],
                            lhsT=xT8[:, :, 128 * a : 128 * (a + 1)],
                            rhs=xT8[:, :, c0 + q0 : c0 + q0 + qw],
                            start=True,
                            stop=True,
                            perf_mode=DR,
                        )
                    else:
                        xTb = st[s]["xTb"]
                        nc.tensor.matmul(
                            out=G[:, q0 : q0 + qw],
                            lhsT=xTb[:, 128 * a : 128 * (a + 1)],
                            rhs=xTb[:, c0 + q0 : c0 + q0 + qw],
                            start=True,
                            stop=True,
                        )
                gs.append((G, c0, w))
                c0 += w
            st[s]["gs"] = gs

        def emit_exp(s, a):
            E = st[s]["E"]
            for (G, c0, w) in st[s]["gs"]:
                o0 = EOF2[a] + (c0 - 128 * C0[a])
                if do('exp'):
                    nc.scalar.activation(
                        out=E[:, o0 : o0 + w], in_=G[:, 0:w], func=AF.Exp, scale=s2r,
                    )
                else:
                    nc.scalar.activation(
                        out=E[:, o0 : o0 + w], in_=G[:, 0:w], func=AF.Copy,
                    )

        def emit_xbar(s, j):
            # mirror row j's off-diag blocks (cols j+1..15) for columns j<CX
            if j >= CX or not do('xbar'):
                return
            E, ET = st[s]["E"], st[s]["ET"]
            nb = (NK - 1) - j
            nc.sync.dma_start_transpose(
                out=ET[:, ET_OFF2[j] : ET_OFF2[j] + nb, :],
                in_=E[:, EOF2[j] + 128 : EOF2[j] + (NK - j) * 128],
            )

        def emit_ya(s, a, qp):
            # Y^T quarter-pass qp (cols [512*qp, 512*(qp+1))) for stationary
            # xs_a. Quarter 0 carries all the xbar-mirror columns (CX=4);
            # quarters use 1 PSUM bank each and run lag-staggered so only
            # two are ever live (pool rotation enforces it).
            E, ET, xs_bf = st[s]["E"], st[s]["ET"], st[s]["xs"]
            yts = st[s]["yt"]
            if yts[qp] is None:
                yts[qp] = psY.tile([64, 512], F32, tag="YT", name=f"YT_{s}_{qp}")
            yt = yts[qp]
            lhsT = xs_bf[:, a, :]
            last = a == NK - 1
            q0 = 512 * qp

            if qp == 0:
                # mirror matmuls (cols j < min(a, CX) <= 4, all in quarter 0)
                for j in range(min(a, CX)):
                    nc.tensor.matmul(
                        out=yt[:, 128 * j : 128 * (j + 1)],
                        lhsT=lhsT,
                        rhs=ET[:, ET_OFF2[j] + (a - j - 1), :],
                        start=False,
                        stop=last and j == CX - 1 and C0[a] >= 4,
                    )
            # strip part inside this quarter: cols [max(q0, 128*C0[a]), q0+512)
            c0 = max(q0, 128 * C0[a])
            w = q0 + 512 - c0
            if w > 0:
                o = EOF2[a] + (c0 - 128 * C0[a])
                nc.tensor.matmul(
                    out=yt[:, c0 - q0 : c0 - q0 + w],
                    lhsT=lhsT,
                    rhs=E[:, o : o + w],
                    start=(a == 0),
                    stop=last,
                )
            if a == 0 and GRAM_FP8:
                # fp8-gram correction terms: += x @ Qc + d @ Pc (this quarter)
                nc.tensor.matmul(
                    out=yt, lhsT=st[s]["Qc"], rhs=st[s]["xTb"][:, q0 : q0 + 512],
                    start=False, stop=False,
                )
                nc.tensor.matmul(
                    out=yt, lhsT=st[s]["Pc"], rhs=st[s]["dT"][:, q0 : q0 + 512],
                    start=False, stop=False,
                )
            if last:
                nc.vector.tensor_copy(
                    out=st[s]["YTsb"][:, q0 : q0 + 512], in_=yt
                )
                yts[qp] = None

        def emit_out(s, g):
            # half g: Y^T 1024-col slab -> natural layout, combine, store
            if not do('all'):
                return
            YTsb, ynat, outsb = st[s]["YTsb"], st[s]["ynat"], st[s]["outsb"]
            f, ax = st[s]["f"], st[s]["ax"]
            nc.sync.dma_start_transpose(
                out=ynat[:, 8 * g : 8 * (g + 1), :],
                in_=YTsb[:, 1024 * g : 1024 * (g + 1)],
            )
            for k in range(8 * g, 8 * (g + 1)):
                nc.vector.scalar_tensor_tensor(
                    out=outsb[:, k, :], in0=ynat[:, k, :], scalar=f[:, k : k + 1],
                    in1=ax[:, k, :], op0=MUL, op1=ADD,
                )
            ov = out_ap[s].rearrange("(p k) c -> p k c", p=128)
            out_legs[g].dma_start(
                out=ov[:, 8 * g : 8 * (g + 1), :], in_=outsb[:, 8 * g : 8 * (g + 1), :]
            )

        # ---- merged pipeline over all samples ----
        emit_front(0)

        if not do('gram'):
            for s in range(1, BPC):
                emit_front(s)
            if do('all'):
                for s in range(BPC):
                    emit_ax(s)
                    st[s]["YTsb"] = sx.tile([64, T], BF16, tag="YTsb", name=f"YTsb_{s}")
                    st[s]["ynat"] = sx.tile([128, NK, C], BF16, tag="ynat", name=f"ynat_{s}")
                    st[s]["outsb"] = sx.tile([128, NK, C], F32, tag="outsb", name=f"outsb_{s}")
                    nc.vector.memset(st[s]["YTsb"], 0.0)
                    for g in range(2):
                        emit_out(s, g)
            return

        QLAG = 9    # step lag between Y quarter-passes (PSUM: <=2 live)
        JMAX = NK - 1 + YSHIFT + 3 * QLAG + 2
        T_END = (BPC - 1) * SOFF + JMAX
        for t in range(-1, T_END + 1):
            for s in range(BPC):
                j = t - s * SOFF
                if j < -1 or j > JMAX:
                    continue
                if j == -1:
                    alloc_main(s)
                    emit_gram(s, 0)
                    continue
                if j < NK:
                    emit_exp(s, j)
                if do('y'):
                    for qp in range(4):
                        ay = j - YSHIFT - QLAG * qp
                        if 0 <= ay < NK:
                            emit_ya(s, ay, qp)
                jg = j + 1
                if jg < NK:
                    emit_gram(s, jg)
                if j < NK:
                    emit_xbar(s, j)
                # hooks: per-sample corr/ax + next sample's front-end
                if j == 0:
                    emit_corr(s)
                elif j == 1:
                    emit_ax(s)
                sn = s + 1
                if sn < BPC:
                    if j == SOFF - 9:
                        emit_cast(sn)
                    elif SOFF - 8 <= j <= SOFF - 5:
                        emit_transp(sn, j - (SOFF - 8))
                        if j == SOFF - 8:
                            emit_sq_mul(sn)
                        elif j == SOFF - 7:
                            emit_sq_reduce(sn)
                    elif j == SOFF - 4:
                        emit_xsf(sn)
                # output halves: half g complete after quarter 2g+1's drain
                if do('y'):
                    if j == NK + YSHIFT + QLAG + 1:
                        emit_out(s, 0)
                    elif j == NK + YSHIFT + 3 * QLAG + 1:
                        emit_out(s, 1)

        if do('all') and not do('y'):
            for s in range(BPC):
                nc.vector.memset(st[s]["YTsb"], 0.0)
                for g in range(2):
                    emit_out(s, g)


_NC_CACHE = {}


def _get_nc(reps=1, stages='all'):
    key = (reps, stages)
    if key not in _NC_CACHE:
        _NC_CACHE[key] = build_nc(reps, stages)
    return _NC_CACHE[key]


def _run(x, r_sigma, margin, trace=False, reps=1, stages='all'):
    nc = _get_nc(reps, stages)
    x = np.ascontiguousarray(np.asarray(x, dtype=np.float32))
    r_sigma = np.ascontiguousarray(np.asarray(r_sigma, dtype=np.float32))
    margin = np.ascontiguousarray(np.asarray(margin, dtype=np.float32))
    in_maps = [
        {
            "x": np.ascontiguousarray(x[c * BPC : (c + 1) * BPC]),
            "r_sigma": r_sigma,
            "margin": margin,
        }
        for c in range(N_CORES)
    ]
    res = run_bass_kernel_spmd(nc, in_maps, core_ids=list(range(N_CORES)), trace=trace)
    out = np.concatenate([res.results[c]["out"] for c in range(N_CORES)], axis=0)
    return out, res


def kernel(x, r_sigma, margin):
    out, _ = _run(x, r_sigma, margin, trace=False)
    return out


# revision 65
# speedup vs baseline: 1.3668x; 1.0850x over previous
"""Trainium2 Bass kernel for nn_K_attention_ex (gaussian-kernel residual attention).

Reference computation (per batch sample b):
    sq_i   = ||x_i||^2
    G      = x @ x^T                      (T,T) gram
    sqdist = relu(sq_i + sq_j - 2 G)
    K      = exp(-sqdist * r + m) * (1 - eye)
    out    = x + K @ x

Algebraic restructuring (exact up to fp rounding):
    K_full = beta * e_i * e_j * exp(2 r g_ij),   e = exp(-r*sq), beta = exp(m)
    out = (1-beta)*x + beta * e ⊙_row ( E @ (e ⊙_row x) ),  E = exp(2 r G)

E is symmetric, so only part of it is materialized. HW-measured costs
(not the cost model!) drove the split:
  * DMA-xbar transpose: ~1.65us fixed per instruction + ~27ns per 16x128
    tile -> mirroring all 120 lower blocks via xbar is ~50us/sample. Only
    columns j < CX are mirrored via xbar (the widest strips, fewest
    instructions per block).
  * ACT exp: ~1.2-2ns/elem on HW. Rows a > CX recompute their lower
    columns [CX, a) directly: each stored E row-strip covers columns
    [min(a,CX), 16), so the Y matmul for stationary xs_a consumes ONE
    contiguous wide rhs (upper + recomputed-lower merged) plus CX small
    mirror matmuls from the xbar'd ET blocks.
  * PE: bf16 matmuls 1cyc/col; rotating 64-col stationaries are ~hidden
    (measured 264ns per N=512 matmul). Wide-N everywhere where possible.
  * Y^T -> natural layout via 2 DMA-xbar transposes; PSUM evacuations on
    DVE (GPSIMD cannot touch PSUM on HW); combine on DVE.

Both samples run in ONE merged software pipeline over global steps
(sample stream offset SOFF), hiding each sample's fill/drain behind the
other's compute.

bf16 operands for gram + Y (output rel err ~4e-3 vs the 2e-2 gate); fp8
gram was rejected: per-row quantization error of x is amplified by the
near-constant positive E into ~3e-2 output error.

Sharding: data-parallel over batch B=16 across 8 NeuronCores (2 samples each).
"""

import numpy as np

import concourse.bass as bass
import concourse.tile as tile
from concourse import bacc, mybir
from concourse.bass_utils import run_bass_kernel_spmd
from concourse.masks import make_identity

F32 = mybir.dt.float32
BF16 = mybir.dt.bfloat16
FP8 = mybir.dt.float8e4
DR = mybir.MatmulPerfMode.DoubleRow
AF = mybir.ActivationFunctionType
MUL = mybir.AluOpType.mult
ADD = mybir.AluOpType.add
SUB = mybir.AluOpType.subtract

B, T, C = 16, 2048, 64
N_CORES = 8
BPC = B // N_CORES          # samples per core
NK = T // 128               # 16 row-blocks of 128
YSHIFT = 2                  # Y stationary lag behind exp/xbar
SOFF = NK - 3               # step offset between sample pipelines (overlap=3)
CX = 4                      # columns j < CX mirrored via DMA-xbar (multiple of 4)
GRAM_FP8 = False            # fp8-DoubleRow gram + rank-64 correction: left
                            # implemented but OFF — measured slower on HW
                            # (DR gave no real speedup; bf16 err is fine)

# E storage: row a holds columns [C0[a], 16) contiguously at EOF2[a].
C0 = [min(a, CX) for a in range(NK)]
WB = [NK - C0[a] for a in range(NK)]          # width in 128-col blocks
EOF2 = []
_o = 0
for _a in range(NK):
    EOF2.append(_o)
    _o += WB[_a] * 128
E_W = _o

# ET: transposed off-diag blocks (rows j < CX only): block (row j, col k>j)
# lands at slot ET_OFF2[j] + (k-j-1); Y stationary a reads slot for (j, a).
ET_OFF2 = []
_o = 0
for _j in range(CX):
    ET_OFF2.append(_o)
    _o += (NK - 1) - _j
N_ET = _o


def build_nc(reps=1, stages='all'):
    nc = bacc.Bacc("TRN2", target_bir_lowering=False, debug=False, num_devices=N_CORES)
    x_in = nc.dram_tensor("x", [BPC, T, C], F32, kind="ExternalInput")
    r_in = nc.dram_tensor("r_sigma", [1], F32, kind="ExternalInput")
    m_in = nc.dram_tensor("margin", [1], F32, kind="ExternalInput")
    o_out = nc.dram_tensor("out", [BPC, T, C], F32, kind="ExternalOutput")

    with tile.TileContext(nc) as tc:
        if reps == 1:
            _body(tc, o_out.ap(), x_in.ap(), r_in.ap(), m_in.ap(), stages)
        else:
            with tc.For_i(0, reps, 1):
                _body(tc, o_out.ap(), x_in.ap(), r_in.ap(), m_in.ap(), stages)
    nc.compile()
    return nc


LEVELS = {'xload': 0, 'xt': 1, 'prep': 2, 'gram': 3, 'exp': 4, 'xbar': 5, 'y': 6, 'all': 7}


def _body(tc, out_ap, x_ap, r_ap, m_ap, stages='all'):
    lvl = LEVELS[stages]
    do = lambda name: lvl >= LEVELS.get(name, 7)
    nc = tc.nc
    with (
        tc.tile_pool(name="consts", bufs=1) as consts,
        tc.tile_pool(name="sx", bufs=2) as sx,
        tc.tile_pool(name="ebig", bufs=2) as ebig,
        tc.tile_pool(name="psG", bufs=3, space="PSUM") as psG,
        tc.tile_pool(name="psY", bufs=2, space="PSUM") as psY,
    ):
        # ---- one-time constants ----
        identb = consts.tile([128, 128], BF16)
        make_identity(nc, identb)
        rb = consts.tile([128, 1], F32)
        nc.gpsimd.dma_start(out=rb, in_=r_ap.to_broadcast((128, 1)))
        mb = consts.tile([128, 1], F32)
        nc.gpsimd.dma_start(out=mb, in_=m_ap.to_broadcast((128, 1)))
        negr = consts.tile([128, 1], F32)
        nc.vector.tensor_scalar_mul(out=negr, in0=rb, scalar1=-1.0)
        s2r = consts.tile([128, 1], F32)
        nc.vector.tensor_scalar_mul(out=s2r, in0=rb, scalar1=2.0)
        beta = consts.tile([128, 1], F32)
        nc.scalar.activation(out=beta, in_=mb, func=AF.Exp)
        alpha = consts.tile([128, 1], F32)  # 1 - beta
        nc.vector.tensor_scalar(
            out=alpha, in0=beta, scalar1=-1.0, scalar2=1.0, op0=MUL, op1=ADD,
        )

        # prefetch all samples' inputs up front
        x_sbs = []
        for s in range(BPC):
            xv = x_ap[s].rearrange("(p k) c -> p k c", p=128)
            x_sb = sx.tile([128, NK, C], F32, tag="x_sb", name=f"x_sb_{s}")
            nc.sync.dma_start(out=x_sb[:, 0:8, :], in_=xv[:, 0:8, :])
            nc.gpsimd.dma_start(out=x_sb[:, 8:NK, :], in_=xv[:, 8:NK, :])
            x_sbs.append(x_sb)

        # ---- per-sample state ----
        st = [
            {
                "x_bf": None, "xTb": None, "xsq": None, "sq": None,
                "f": None, "xs": None, "ax": None,
                "E": None, "ET": None, "YTsb": None, "ynat": None,
                "outsb": None, "yt": [None] * 4, "gs": None,
                "xT8": None, "dT": None, "x8n": None, "dn": None,
                "Pc": None, "Qc": None,
            }
            for _ in range(BPC)
        ]
        out_legs = [nc.sync, nc.gpsimd]

        # ---- front-end emitters ----
        def emit_sq_mul(s):
            if not do('prep'):
                return
            xsq = sx.tile([128, NK, C], F32, tag="xsq", name=f"xsq_{s}")
            nc.vector.tensor_mul(xsq, x_sbs[s], x_sbs[s])
            st[s]["xsq"] = xsq

        def emit_sq_reduce(s):
            if not do('prep'):
                return
            sq = sx.tile([128, NK], F32, tag="sq", name=f"sq_{s}")
            nc.vector.reduce_sum(out=sq, in_=st[s]["xsq"], axis=mybir.AxisListType.X)
            st[s]["sq"] = sq

        def emit_cast(s):
            x_bf = sx.tile([128, NK, C], BF16, tag="x_bf", name=f"x_bf_{s}")
            # two halves, following the two input-DMA legs (subtile deps)
            nc.vector.tensor_copy(out=x_bf[:, 0:8, :], in_=x_sbs[s][:, 0:8, :])
            nc.vector.tensor_copy(out=x_bf[:, 8:NK, :], in_=x_sbs[s][:, 8:NK, :])
            st[s]["x_bf"] = x_bf
            st[s]["xTb"] = sx.tile([64, T], BF16, tag="xTb", name=f"xTb_{s}")

        def emit_transp(s, g):
            if not do('xt'):
                return
            x_bf, xTb = st[s]["x_bf"], st[s]["xTb"]
            if g == 0 and GRAM_FP8:
                st[s]["xT8"] = sx.tile([64, 2, T], FP8, tag="xT8", name=f"xT8_{s}")
                nc.gpsimd.memset(st[s]["xT8"][:, 1, :], 0.0)
                st[s]["dT"] = sx.tile([64, T], BF16, tag="dT", name=f"dT_{s}")
            xtr = psG.tile([64, 4, 128], BF16, tag="G", name=f"xtr_{s}_{g}")
            for kk in range(4):
                k = 4 * g + kk
                nc.tensor.transpose(
                    out=xtr[:, kk, :], in_=x_bf[:, k, :], identity=identb
                )
            sl = slice(512 * g, 512 * (g + 1))
            nc.vector.tensor_copy(
                out=xTb[:, sl], in_=xtr.rearrange("p a b -> p (a b)")
            )
            if GRAM_FP8:
                # fp8 mirror of x^T (DoubleRow gram operand) + bf16 residual
                nc.vector.tensor_copy(
                    out=st[s]["xT8"][:, 0, sl], in_=xtr.rearrange("p a b -> p (a b)")
                )
                nc.vector.tensor_sub(
                    st[s]["dT"][:, sl], xTb[:, sl], st[s]["xT8"][:, 0, sl]
                )

        def emit_xsf(s):
            if not do('prep'):
                return
            e = sx.tile([128, NK], F32, tag="e", name=f"e_{s}")
            nc.scalar.activation(out=e, in_=st[s]["sq"], func=AF.Exp, scale=negr)
            f = sx.tile([128, NK], F32, tag="f", name=f"f_{s}")
            nc.vector.tensor_scalar_mul(out=f, in0=e, scalar1=beta)
            xs_bf = sx.tile([128, NK, C], BF16, tag="xs_bf", name=f"xs_bf_{s}")
            for k in range(NK):
                nc.vector.tensor_scalar_mul(
                    out=xs_bf[:, k, :], in0=st[s]["x_bf"][:, k, :],
                    scalar1=e[:, k : k + 1],
                )
            st[s]["f"], st[s]["xs"] = f, xs_bf

        def emit_ax(s):
            if not do('prep'):
                return
            ax = sx.tile([128, NK, C], F32, tag="ax", name=f"ax_{s}")
            nc.vector.tensor_scalar_mul(out=ax, in0=x_sbs[s], scalar1=alpha)
            st[s]["ax"] = ax

        def emit_corr(s):
            # Rank-64 correction for the fp8 gram: with d = x - fp8(x),
            #   Y_true - Y_fp8 ~= 2r * (x @ Q + d @ (P - Q)),
            #   P = x^T xs,  Q = d^T xs   (64x64 each)
            # This cancels the systematic per-row quantization error that
            # the near-constant positive E amplifies (2.6e-2 -> 4.6e-3).
            if not do('y') or not GRAM_FP8:
                return
            x_bf, xs_bf = st[s]["x_bf"], st[s]["xs"]
            x8n = sx.tile([128, NK, C], FP8, tag="x8n", name=f"x8n_{s}", bufs=1)
            nc.vector.tensor_copy(out=x8n, in_=x_bf)
            dn = sx.tile([128, NK, C], BF16, tag="dn", name=f"dn_{s}", bufs=1)
            nc.vector.tensor_sub(dn, x_bf, x8n)
            P = psG.tile([64, 64], F32, tag="G", name=f"P_{s}")
            for k in range(NK):
                nc.tensor.matmul(
                    out=P, lhsT=x_bf[:, k, :], rhs=xs_bf[:, k, :],
                    start=(k == 0), stop=(k == NK - 1),
                )
            Q = psG.tile([64, 64], F32, tag="G", name=f"Q_{s}")
            for k in range(NK):
                nc.tensor.matmul(
                    out=Q, lhsT=dn[:, k, :], rhs=xs_bf[:, k, :],
                    start=(k == 0), stop=(k == NK - 1),
                )
            Qc = sx.tile([64, 64], BF16, tag="Qc", name=f"Qc_{s}")
            nc.vector.tensor_scalar_mul(out=Qc, in0=Q, scalar1=s2r[:64])
            Pc = sx.tile([64, 64], BF16, tag="Pc", name=f"Pc_{s}")
            nc.vector.scalar_tensor_tensor(
                out=Pc, in0=P, scalar=s2r[:64], in1=Qc, op0=MUL, op1=SUB,
            )
            st[s]["Pc"], st[s]["Qc"] = Pc, Qc

        def emit_front(s):
            emit_cast(s)
            emit_transp(s, 0)
            emit_transp(s, 1)
            emit_sq_mul(s)
            emit_sq_reduce(s)
            emit_transp(s, 2)
            emit_transp(s, 3)
            emit_xsf(s)

        def alloc_main(s):
            st[s]["E"] = ebig.tile([128, E_W], BF16, tag="E", name=f"E_{s}")
            st[s]["ET"] = ebig.tile(
                [128, N_ET, 128], BF16, tag="ET", name=f"ET_{s}", bufs=1
            )
            st[s]["YTsb"] = sx.tile([64, T], BF16, tag="YTsb", name=f"YTsb_{s}")
            st[s]["ynat"] = sx.tile([128, NK, C], BF16, tag="ynat", name=f"ynat_{s}")
            st[s]["outsb"] = sx.tile([128, NK, C], F32, tag="outsb", name=f"outsb_{s}")

        # ---- main-pipeline emitters ----
        def emit_gram(s, a):
            """G row-strip a (fp8 DoubleRow, 0.5 PE cyc/col) over columns
            [128*C0[a], 2048).

            1536-col G tiles (3 bank-sized sub-matmuls each) so the exp
            reads 1536-wide chunks: ACT per-instruction overhead measured
            ~650ns, so wide chunks substantially cut the HW exp cost."""
            gs = []
            c0 = 128 * C0[a]
            while c0 < T:
                w = min(1024, T - c0)
                G = psG.tile([128, 1024], F32, tag="G", name=f"G_{s}_{a}_{c0}")
                for q0 in range(0, w, 512):
                    qw = min(512, w - q0)
                    if GRAM_FP8:
                        xT8 = st[s]["xT8"]
                        nc.tensor.matmul(
                            out=G[:, q0 : q0 + qw],
                            lhsT=xT8[:, :, 128 * a : 128 * (a + 1)],
                            rhs=xT8[:, :, c0 + q0 : c0 + q0 + qw],
                            start=True,
                            stop=True,
                            perf_mode=DR,
                        )
                    else:
                        xTb = st[s]["xTb"]
                        nc.tensor.matmul(
                            out=G[:, q0 : q0 + qw],
                            lhsT=xTb[:, 128 * a : 128 * (a + 1)],
                            rhs=xTb[:, c0 + q0 : c0 + q0 + qw],
                            start=True,
                            stop=True,
                        )
                gs.append((G, c0, w))
                c0 += w
            st[s]["gs"] = gs

        def emit_exp(s, a):
            E = st[s]["E"]
            for (G, c0, w) in st[s]["gs"]:
                o0 = EOF2[a] + (c0 - 128 * C0[a])
                if do('exp'):
                    nc.scalar.activation(
                        out=E[:, o0 : o0 + w], in_=G[:, 0:w], func=AF.Exp, scale=s2r,
                    )
                else:
                    nc.scalar.activation(
                        out=E[:, o0 : o0 + w], in_=G[:, 0:w], func=AF.Copy,
                    )

        def emit_xbar(s, j):
            # mirror row j's off-diag blocks (cols j+1..15) for columns j<CX
            if j >= CX or not do('xbar'):
                return
            E, ET = st[s]["E"], st[s]["ET"]
            nb = (NK - 1) - j
            nc.sync.dma_start_transpose(
                out=ET[:, ET_OFF2[j] : ET_OFF2[j] + nb, :],
                in_=E[:, EOF2[j] + 128 : EOF2[j] + (NK - j) * 128],
            )

        def emit_ya(s, a, qp):
            # Y^T quarter-pass qp (cols [512*qp, 512*(qp+1))) for stationary
            # xs_a. Quarter 0 carries all the xbar-mirror columns (CX=4);
            # quarters use 1 PSUM bank each and run lag-staggered so only
            # two are ever live (pool rotation enforces it).
            E, ET, xs_bf = st[s]["E"], st[s]["ET"], st[s]["xs"]
            yts = st[s]["yt"]
            if yts[qp] is None:
                yts[qp] = psY.tile([64, 512], F32, tag="YT", name=f"YT_{s}_{qp}")
            yt = yts[qp]
            lhsT = xs_bf[:, a, :]
            last = a == NK - 1
            q0 = 512 * qp

            if qp == 0:
                # mirror matmuls (cols j < min(a, CX) <= 4, all in quarter 0)
                for j in range(min(a, CX)):
                    nc.tensor.matmul(
                        out=yt[:, 128 * j : 128 * (j + 1)],
                        lhsT=lhsT,
                        rhs=ET[:, ET_OFF2[j] + (a - j - 1), :],
                        start=False,
                        stop=last and j == CX - 1 and C0[a] >= 4,
                    )
            # strip part inside this quarter: cols [max(q0, 128*C0[a]), q0+512)
            c0 = max(q0, 128 * C0[a])
            w = q0 + 512 - c0
            if w > 0:
                o = EOF2[a] + (c0 - 128 * C0[a])
                nc.tensor.matmul(
                    out=yt[:, c0 - q0 : c0 - q0 + w],
                    lhsT=lhsT,
                    rhs=E[:, o : o + w],
                    start=(a == 0),
                    stop=last,
                )
            if a == 0 and GRAM_FP8:
                # fp8-gram correction terms: += x @ Qc + d @ Pc (this quarter)
                nc.tensor.matmul(
                    out=yt, lhsT=st[s]["Qc"], rhs=st[s]["xTb"][:, q0 : q0 + 512],
                    start=False, stop=False,
                )
                nc.tensor.matmul(
                    out=yt, lhsT=st[s]["Pc"], rhs=st[s]["dT"][:, q0 : q0 + 512],
                    start=False, stop=False,
                )
            if last:
                nc.vector.tensor_copy(
                    out=st[s]["YTsb"][:, q0 : q0 + 512], in_=yt
                )
                yts[qp] = None

        def emit_out(s, g):
            # half g: Y^T 1024-col slab -> natural layout, combine, store
            if not do('all'):
                return
            YTsb, ynat, outsb = st[s]["YTsb"], st[s]["ynat"], st[s]["outsb"]
            f, ax = st[s]["f"], st[s]["ax"]
            nc.sync.dma_start_transpose(
                out=ynat[:, 8 * g : 8 * (g + 1), :],
                in_=YTsb[:, 1024 * g : 1024 * (g + 1)],
            )
            for k in range(8 * g, 8 * (g + 1)):
                nc.vector.scalar_tensor_tensor(
                    out=outsb[:, k, :], in0=ynat[:, k, :], scalar=f[:, k : k + 1],
                    in1=ax[:, k, :], op0=MUL, op1=ADD,
                )
            ov = out_ap[s].rearrange("(p k) c -> p k c", p=128)
            out_legs[g].dma_start(
                out=ov[:, 8 * g : 8 * (g + 1), :], in_=outsb[:, 8 * g : 8 * (g + 1), :]
            )

        # ---- merged pipeline over all samples ----
        emit_front(0)

        if not do('gram'):
            for s in range(1, BPC):
                emit_front(s)
            if do('all'):
                for s in range(BPC):
                    emit_ax(s)
                    st[s]["YTsb"] = sx.tile([64, T], BF16, tag="YTsb", name=f"YTsb_{s}")
                    st[s]["ynat"] = sx.tile([128, NK, C], BF16, tag="ynat", name=f"ynat_{s}")
                    st[s]["outsb"] = sx.tile([128, NK, C], F32, tag="outsb", name=f"outsb_{s}")
                    nc.vector.memset(st[s]["YTsb"], 0.0)
                    for g in range(2):
                        emit_out(s, g)
            return

        QLAG = 8    # step lag between Y quarter-passes (PSUM: <=2 live)
        JMAX = NK - 1 + YSHIFT + 3 * QLAG + 2
        T_END = (BPC - 1) * SOFF + JMAX
        for t in range(-1, T_END + 1):
            for s in range(BPC):
                j = t - s * SOFF
                if j < -1 or j > JMAX:
                    continue
                if j == -1:
                    alloc_main(s)
                    emit_gram(s, 0)
                    continue
                if j < NK:
                    emit_exp(s, j)
                if do('y'):
                    for qp in range(4):
                        ay = j - YSHIFT - QLAG * qp
                        if 0 <= ay < NK:
                            emit_ya(s, ay, qp)
                jg = j + 1
                if jg < NK:
                    emit_gram(s, jg)
                if j < NK:
                    emit_xbar(s, j)
                # hooks: per-sample corr/ax + next sample's front-end
                if j == 0:
                    emit_corr(s)
                elif j == 1:
                    emit_ax(s)
                sn = s + 1
                if sn < BPC:
                    if j == SOFF - 9:
                        emit_cast(sn)
                    elif SOFF - 8 <= j <= SOFF - 5:
                        emit_transp(sn, j - (SOFF - 8))
                        if j == SOFF - 8:
                            emit_sq_mul(sn)
                        elif j == SOFF - 7:
                            emit_sq_reduce(sn)
                    elif j == SOFF - 4:
                        emit_xsf(sn)
                # output halves: half g complete after quarter 2g+1's drain
                if do('y'):
                    if j == NK + YSHIFT + QLAG + 1:
                        emit_out(s, 0)
                    elif j == NK + YSHIFT + 3 * QLAG + 1:
                        emit_out(s, 1)

        if do('all') and not do('y'):
            for s in range(BPC):
                nc.vector.memset(st[s]["YTsb"], 0.0)
                for g in range(2):
                    emit_out(s, g)


_NC_CACHE = {}


def _get_nc(reps=1, stages='all'):
    key = (reps, stages)
    if key not in _NC_CACHE:
        _NC_CACHE[key] = build_nc(reps, stages)
    return _NC_CACHE[key]


def _run(x, r_sigma, margin, trace=False, reps=1, stages='all'):
    nc = _get_nc(reps, stages)
    x = np.ascontiguousarray(np.asarray(x, dtype=np.float32))
    r_sigma = np.ascontiguousarray(np.asarray(r_sigma, dtype=np.float32))
    margin = np.ascontiguousarray(np.asarray(margin, dtype=np.float32))
    in_maps = [
        {
            "x": np.ascontiguousarray(x[c * BPC : (c + 1) * BPC]),
            "r_sigma": r_sigma,
            "margin": margin,
        }
        for c in range(N_CORES)
    ]
    res = run_bass_kernel_spmd(nc, in_maps, core_ids=list(range(N_CORES)), trace=trace)
    out = np.concatenate([res.results[c]["out"] for c in range(N_CORES)], axis=0)
    return out, res


def kernel(x, r_sigma, margin):
    out, _ = _run(x, r_sigma, margin, trace=False)
    return out
